# revision 30
# baseline (speedup 1.0000x reference)
"""Edge-parallel GNN discriminator kernel for 8 TRN2 NeuronCores.

Computes Y[e] = sigmoid(w * dot(Z[src[e]], Z[dst[e]]) + b) for E edges.

Strategy (edge-parallel, per the sharding hint):
  - Each of the 8 cores holds a full Z replica in HBM and processes E/8 edges.
  - Z is converted to fp16 host-side (dot rel-err ~5e-4, well under the 2e-2
    gate): halves gather DMA traffic and doubles DVE throughput.
  - Row gather uses the Anthropic dma_gather extended instruction (one 256B
    descriptor per row, descriptors spread over all 16 SDMA engines, Q7-pair
    descriptor generation). dma_gather takes int16 indices (< 32768), but
    N = 50000 rows: view Z as 25000 row-PAIRS (stride 512B) and gather with
    q = row >> 1 from a base offset selected by row parity. Edges are sorted
    host-side into 4 segments by (src parity, dst parity) so each tile's two
    gathers use compile-time base offsets; the host inverse-permutes outputs.
  - dma_gather writes slot s -> partition s%128, block s//128. Per tile:
    zs, zd [128, T/128, 128] fp16; DVE multiplies (fp16 2x mode) and reduces
    via a binary tree of strided tensor_tensor adds (tensor_reduce has no
    16-bit fast path; the tree runs almost entirely in 2x mode), final 2-way
    add into fp32 dots; one ACT sigmoid (scale=w, bias=b) over the
    accumulated result; contiguous DMA out.
  - BIG_T=896 is the largest tile compatible with single_packet=True (a
    gather's per-engine descriptors must fit one 64-desc packet); bigger
    tiles with single_packet=False measured substantially slower.
  - All cores run one SPMD NEFF: segment capacities are the max over cores,
    padded to tile granularity with dummy slots (index 0) that are dropped
    host-side.
"""

import numpy as np

import concourse.bacc as bacc
import concourse.mybir as mybir
from concourse.tile import TileContext
from concourse.bass_utils import run_bass_kernel_spmd

N_CORES = 8
P = 128
D = 128
# With single_packet=True a gather's per-SDMA-engine descriptors must fit one
# packet (<=64 descs): T/16 + flush + sem <= 64 -> max T (mult of 128) = 896.
# single_packet=False lifts the cap but measured ~50% slower; keep 896/True.
BIG_T = 896
SMALL_T = 128
DMA_SCRATCH = 16384
NUM_QUEUES = 4  # ucode MAX_SWDGE_QUEUES; q=2 measured ~2.6x slower
SINGLE_PACKET = True
GATHER_BUFS = 8
# Lane->queue map. Paired tiles alternate small (T/2-desc) pair gathers and
# big (T-desc) dst gathers on consecutive lanes; plain L%4 then sends all
# small gathers to queues 0/2 and all big ones to 1/3 (1:2 desc-gen load
# imbalance on the bottleneck Q7 pairs). (L//2)%4 gives each tile's two
# gathers to one queue and round-robins tiles: measured max queue load
# drops 99.5k -> 76.8k descs (-23%).
QUEUE_FROM_LANE = lambda L: (L // 2) % 4  # noqa: E731
GDT = mybir.dt.float16
REDUCE = "tree"  # "tree" (fp16 2x tensor_tensor tree) | "reduce" (tensor_reduce)
SORT_SRC = True  # sort each parity segment by src row for HBM locality
# Pair edges whose src rows are the two halves of one 512B row-pair: one
# gather descriptor serves both edges (~62% of edges pair; 1.71 descs/edge
# instead of 2). Probe measured desc count as the dominant gather cost;
# same-session A/B: 838us vs 1347us (38% faster in that session).
PAIR_SRC = True
# Edges per paired tile: 768 pair-descs (48/engine, inside the 64-desc
# single-packet cap) + two 768-desc per-edge gathers. All gathers are then
# equal-weight (768 descs), so queue balance is exact, and instructions per
# edge drop 25% vs PAIR_T=768 (the ~1us fixed Q7 launch is significant).
PAIR_T = 1536
PAIR_DST = True  # second-order: pair first-order leftovers by dst row-pair
# Deal q-chunks vs strides to cores: chunked (contiguous q windows) measured
# ~12% SLOWER on the same loaded NEFF — 16 SDMA engines round-robin the desc
# stream, so a narrow address window serializes on few HBM channels while
# striding engages all channels. Keep strided.
CHUNK_DEAL = False


def _plan_tiles(n):
    """Tile sizes (each a multiple of SMALL_T) covering >= n slots."""
    n = max(n, SMALL_T)
    ts = [BIG_T] * (n // BIG_T)
    rem = n - (n // BIG_T) * BIG_T
    if rem:
        ts.append(((rem + SMALL_T - 1) // SMALL_T) * SMALL_T)
    return ts


def _build(nc, n_nodes, tiles, slots, gather_bufs=GATHER_BUFS, schedule=None,
           compute=True):
    """tiles: list of (src_parity, dst_parity, T) in slot order.

    schedule: optional explicit list of (ps, pd, T, slot_base); defaults to
    the running-offset schedule implied by `tiles` (used by perf rigs to
    repeat the tile schedule)."""
    ncols = slots // 16
    rcols = slots // 128
    Z = nc.dram_tensor("Z", [n_nodes, D], GDT, kind="ExternalInput")
    ISRC = nc.dram_tensor("isrc", [16, ncols], mybir.dt.int16, kind="ExternalInput")
    IDST = nc.dram_tensor("idst", [16, ncols], mybir.dt.int16, kind="ExternalInput")
    WB = nc.dram_tensor("wb", [P, 2], mybir.dt.float32, kind="ExternalInput")
    Y = nc.dram_tensor("y", [P, rcols], mybir.dt.float32, kind="ExternalOutput")

    # Z as row-pairs: [25000, 256] fp16; parity r selects a 128-elem offset.
    zview = Z[:].rearrange("(q r) d -> q (r d)", r=2)
    TBM = max(BIG_T, PAIR_T if PAIR_SRC else 0) // 128

    with TileContext(nc) as tc:
        with (
            tc.tile_pool(name="idxp", bufs=1) as idx_pool,
            tc.tile_pool(name="gath", bufs=gather_bufs) as gpool,
            tc.tile_pool(name="misc", bufs=1) as mpool,
        ):
            isrc_sb = idx_pool.tile([P, ncols], mybir.dt.int16, tag="isrc")
            idst_sb = idx_pool.tile([P, ncols], mybir.dt.int16, tag="idst")
            # Each Q7 core reads idxs from its own 16-partition window:
            # replicate the wrapped idx block into all 8 groups.
            for grp in range(8):
                nc.sync.dma_start(
                    out=isrc_sb[grp * 16:(grp + 1) * 16, :], in_=ISRC[:]
                )
                nc.sync.dma_start(
                    out=idst_sb[grp * 16:(grp + 1) * 16, :], in_=IDST[:]
                )

            wb_sb = mpool.tile([P, 2], mybir.dt.float32, tag="wb")
            nc.sync.dma_start(out=wb_sb[:], in_=WB[:])
            wb_dve = mpool.tile([P, 2], mybir.dt.float32, tag="wbd")
            nc.vector.tensor_copy(out=wb_dve[:], in_=wb_sb[:])

            res = mpool.tile([P, rcols], mybir.dt.float32, tag="res")
            if not compute:
                nc.vector.memset(res[:], 0)

            if schedule is None:
                schedule = []
                slot_base = 0
                for (ps, pd, T) in tiles:
                    schedule.append((ps, pd, T, slot_base))
                    slot_base += T
            for ti, (ps, pd, T, slot_base) in enumerate(schedule):
                Tb = T // 128
                icol0 = slot_base // 16
                rcol0 = slot_base // 128
                zs = gpool.tile([P, TBM * D], GDT, tag="zs")
                zd = gpool.tile([P, TBM * D], GDT, tag="zd")
                if ps < 0:
                    # Paired tile: one 512B descriptor per PAIR of edges
                    # whose src rows (ps==-1) or dst rows (ps==-2) are the
                    # two halves of one row-pair q. The flattened
                    # [p, 2j+h -> b] layout coincides with the per-edge
                    # layout, so compute below is unchanged.
                    pbuf, pidx = ((zs, isrc_sb) if ps == -1 else
                                  (zd, idst_sb))
                    ebuf, eidx = ((zd, idst_sb) if ps == -1 else
                                  (zs, isrc_sb))
                    npairs = T // 2
                    nc.gpsimd.dma_gather(
                        out_ap=pbuf[:, : Tb * D].rearrange(
                            "p (k d) -> p k d", d=2 * D),
                        in_ap=zview[:],
                        idxs_ap=pidx[:, icol0:icol0 + npairs // 16],
                        num_idxs=npairs,
                        num_idxs_reg=npairs,
                        elem_size=2 * D,
                        elem_step=2 * D,
                        queue_num=0,
                        single_packet=SINGLE_PACKET,
                    )
                    gathers = [(ebuf, pd, eidx)]
                else:
                    gathers = [(zs, ps, isrc_sb), (zd, pd, idst_sb)]
                for buf, par, itile in gathers:
                    # Per-edge gathers in <=896-idx pieces (single-packet cap
                    # is 64 descs/engine ~ 1008 idxs); 768-size pieces for
                    # large tiles keep every gather equal-weight.
                    pieces = [(0, T)] if T <= 896 else [
                        (e0, min(768, T - e0)) for e0 in range(0, T, 768)]
                    for e0, n in pieces:
                        nc.gpsimd.dma_gather(
                            out_ap=buf[:, (e0 // 128) * D:
                                       ((e0 + n) // 128) * D].rearrange(
                                "p (k d) -> p k d", d=D),
                            in_ap=zview[:, par * D:(par + 1) * D],
                            idxs_ap=itile[:, (slot_base + e0) // 16:
                                          (slot_base + e0 + n) // 16],
                            num_idxs=n,
                            num_idxs_reg=n,
                            elem_size=D,
                            elem_step=2 * D,
                            queue_num=0,
                            single_packet=SINGLE_PACKET,
                        )
                if compute:
                    prod = gpool.tile([P, TBM * D], GDT, tag="prod")
                    nc.vector.tensor_tensor(
                        out=prod[:, : Tb * D],
                        in0=zs[:, : Tb * D],
                        in1=zd[:, : Tb * D],
                        op=mybir.AluOpType.mult,
                    )
                    if REDUCE == "reduce":
                        nc.vector.reduce_sum(
                            out=res[:, rcol0:rcol0 + Tb],
                            in_=prod[:, : Tb * D].rearrange(
                                "p (k d) -> p k d", d=D),
                            axis=mybir.AxisListType.X,
                        )
                    else:
                        # Binary-tree reduce over the 128 dims: fp16 strided
                        # adds (2x DVE mode) ping-ponging through the dead
                        # zs/zd bufs, final 2->1 add lands in fp32 res.
                        cur, w = prod, D
                        scratch = [zs, zd]
                        lvl = 0
                        while w > 2:
                            h = w // 2
                            nxt = scratch[lvl % 2]
                            cv = cur[:, : Tb * w].rearrange(
                                "p (k d) -> p k d", d=w)
                            nc.vector.tensor_tensor(
                                out=nxt[:, : Tb * h],
                                in0=cv[:, :, 0:h],
                                in1=cv[:, :, h:w],
                                op=mybir.AluOpType.add,
                            )
                            cur, w = nxt, h
                            lvl += 1
                        cv = cur[:, : Tb * 2].rearrange("p (k d) -> p k d", d=2)
                        nc.vector.tensor_tensor(
                            out=res[:, rcol0:rcol0 + Tb],
                            in0=cv[:, :, 0:1],
                            in1=cv[:, :, 1:2],
                            op=mybir.AluOpType.add,
                        )

            yt = mpool.tile([P, rcols], mybir.dt.float32, tag="yt")
            nc.scalar.activation(
                out=yt[:],
                in_=res[:],
                func=mybir.ActivationFunctionType.Sigmoid,
                bias=wb_dve[:, 1:2],
                scale=wb_dve[:, 0:1],
            )
            nc.sync.dma_start(out=Y[:], in_=yt[:])
    return nc


def _spread_gather_queues(nc):
    """Post-schedule: spread dma_gather descriptor generation over the 4 Q7
    core pairs. Tile assigns each SWDGE DMA a DMASW{L} completion-sem lane in
    scheduled order; a lane must be fed by a single queue, so queue = L %
    NUM_QUEUES keeps the lane->queue map consistent while rotating work
    across queues."""
    if NUM_QUEUES == 1:
        return
    for inst in nc.inst_map.values():
        if not isinstance(inst, mybir.InstDMAGatherAnt):
            continue
        si = inst.sync_info
        if si is None or not si.on_update:
            continue
        name = si.on_update[0].ant_name or ""
        if name.startswith("DMASW"):
            lane = int(name[5:].split("_")[0])
            fn = QUEUE_FROM_LANE or (lambda L: L % NUM_QUEUES)
            inst.queue_num = fn(lane)


def _take_first_m(q_sorted, m):
    """Mask selecting, for each value v, the first m[v] entries of the
    ascending-sorted array q_sorted."""
    starts = np.searchsorted(q_sorted, np.arange(len(m)))
    rank = np.arange(len(q_sorted)) - starts[q_sorted]
    return rank < m[q_sorted]


def _plan_pair_tiles(npairs):
    """Pair-tile sizes (in pairs, multiples of 128) covering >= npairs."""
    npairs = max(npairs, 128)
    big = PAIR_T // 2
    ts = [big] * (npairs // big)
    rem = npairs - (npairs // big) * big
    if rem:
        ts.append(((rem + 127) // 128) * 128)
    return ts


def _prepare(Z, edge_index, w, b):
    """Host-side sharding/packing. Returns (in_maps, s2e_list, tiles, slots, E, Nn)."""
    Z = np.asarray(Z, dtype=np.float32).astype(np.float16)
    ei = np.asarray(edge_index)
    w = np.asarray(w, dtype=np.float32).reshape(-1)[0]
    b = np.asarray(b, dtype=np.float32).reshape(-1)[0]
    n_nodes = Z.shape[0]
    E = ei.shape[1]
    src_all = ei[0].astype(np.int32)
    dst_all = ei[1].astype(np.int32)
    per_core = (E + N_CORES - 1) // N_CORES
    if PAIR_SRC:
        return _prepare_paired(Z, src_all, dst_all, w, b, per_core, E, n_nodes)

    cores = []
    seg_counts = np.zeros((N_CORES, 4), np.int64)
    for c in range(N_CORES):
        lo = c * per_core
        hi = min(E, lo + per_core)
        s = src_all[lo:hi]
        d = dst_all[lo:hi]
        g = ((s & 1) << 1) | (d & 1)
        if SORT_SRC:
            # Within each parity segment, order edges by src row so the src
            # gather's descriptor stream walks HBM mostly monotonically
            # (DRAM row-buffer locality; repeated rows coalesce).
            perm = np.lexsort((s >> 1, g))
        else:
            perm = np.argsort(g, kind="stable")
        cores.append((lo, s, d, g, perm))
        seg_counts[c] = np.bincount(g, minlength=4)

    caps = seg_counts.max(axis=0)
    seg_tiles = [_plan_tiles(int(caps[gg])) for gg in range(4)]
    seg_cap = [sum(ts) for ts in seg_tiles]
    slots = int(sum(seg_cap))
    tiles = []
    for gg in range(4):
        tiles += [(gg >> 1, gg & 1, T) for T in seg_tiles[gg]]

    wb = np.stack([np.full(P, w), np.full(P, b)], axis=1).astype(np.float32)
    in_maps = []
    s2e_list = []
    for c in range(N_CORES):
        lo, s, d, g, perm = cores[c]
        qs = np.zeros(slots, np.int16)
        qd = np.zeros(slots, np.int16)
        s2e = np.full(slots, -1, np.int64)
        off = 0
        gp = g[perm]
        for gg in range(4):
            e = perm[gp == gg]
            n = len(e)
            qs[off:off + n] = (s[e] >> 1).astype(np.int16)
            qd[off:off + n] = (d[e] >> 1).astype(np.int16)
            s2e[off:off + n] = lo + e
            off += seg_cap[gg]
        in_maps.append({
            "Z": Z,
            "isrc": np.ascontiguousarray(qs.reshape(-1, 16).T),
            "idst": np.ascontiguousarray(qd.reshape(-1, 16).T),
            "wb": wb,
        })
        s2e_list.append(s2e)
    return in_maps, s2e_list, tiles, slots, E, n_nodes


def _prepare_paired(Z, src_all, dst_all, w, b, per_core, E, n_nodes):
    """Packing with src-pair descriptors, pooled globally.

    Pairing is done over the WHOLE edge list (not per core): pooling the
    parity imbalance over 8x the edges raises the pair rate from ~62% to
    ~86%. Pairs/leftovers are then dealt round-robin to cores, which also
    equalizes per-core segment sizes (near-zero padding).

    Segments per core: P0, P1 (paired edges, by dst parity) then U00..U11
    (unpaired, by (src parity, dst parity)). In a P segment, pair j of a
    tile occupies slots b*128+p for b = 2*(j//128)+h, p = j%128, h = which
    half of row-pair q the edge's src row is."""
    nq = (n_nodes + 1) // 2
    s = src_all
    d = dst_all
    q = s >> 1
    spar = s & 1
    dpar = d & 1
    # Global pairing per dst-parity group; pair lists stay q-sorted, so
    # dealing round-robin hands each core a q-sorted (HBM-local) list.
    pairs_gd = []
    un_gg = []
    for gd in (0, 1):
        sel = np.nonzero(dpar == gd)[0]
        e0 = sel[spar[sel] == 0]
        e1 = sel[spar[sel] == 1]
        e0 = e0[np.argsort(q[e0], kind="stable")]
        e1 = e1[np.argsort(q[e1], kind="stable")]
        c0 = np.bincount(q[e0], minlength=nq)
        c1 = np.bincount(q[e1], minlength=nq)
        m = np.minimum(c0, c1)
        m0 = _take_first_m(q[e0], m)
        m1 = _take_first_m(q[e1], m)
        pairs_gd.append((e0[m0], e1[m1]))  # aligned by q
        un_gd = (e0[~m0], e1[~m1])
        for gs in (0, 1):
            un_gg.append([gs, gd, un_gd[gs]])

    # Second-order: pair the leftovers by dst row-pair (roles swapped; the
    # per-edge gather is then the SRC side, so pool across gd per gs).
    qd_all = d >> 1
    pairs2_gs = []
    if PAIR_DST:
        left = {(gs, gd): e for gs, gd, e in un_gg}
        un_gg = []
        for gs in (0, 1):
            f0 = left[(gs, 0)]  # dst parity 0 -> half h=0
            f1 = left[(gs, 1)]  # dst parity 1 -> half h=1
            f0 = f0[np.argsort(qd_all[f0], kind="stable")]
            f1 = f1[np.argsort(qd_all[f1], kind="stable")]
            c0 = np.bincount(qd_all[f0], minlength=nq)
            c1 = np.bincount(qd_all[f1], minlength=nq)
            m = np.minimum(c0, c1)
            m0 = _take_first_m(qd_all[f0], m)
            m1 = _take_first_m(qd_all[f1], m)
            pairs2_gs.append((f0[m0], f1[m1]))  # aligned by qd
            un_gg.append([gs, 0, f0[~m0]])
            un_gg.append([gs, 1, f1[~m1]])

    if SORT_SRC:
        for ent in un_gg:
            e = ent[2]
            ent[2] = e[np.argsort(q[e], kind="stable")]

    pair_caps = [(len(p0) + N_CORES - 1) // N_CORES for p0, p1 in pairs_gd]
    pair_tiles = [_plan_pair_tiles(int(cap)) for cap in pair_caps]
    pair2_caps = [(len(p0) + N_CORES - 1) // N_CORES for p0, p1 in pairs2_gs]
    pair2_tiles = [_plan_pair_tiles(int(cap)) for cap in pair2_caps]
    un_caps = {}
    un_tiles = [None] * 4
    for gs, gd, e in un_gg:
        gg = gs * 2 + gd
        un_caps[gg] = un_caps.get(gg, 0) + (len(e) + N_CORES - 1) // N_CORES
    for gg in range(4):
        un_tiles[gg] = _plan_tiles(int(un_caps.get(gg, 0)))

    tiles = []
    for gd in (0, 1):
        tiles += [(-1, gd, 2 * tp) for tp in pair_tiles[gd]]
    for gs in range(len(pairs2_gs)):
        tiles += [(-2, gs, 2 * tp) for tp in pair2_tiles[gs]]
    for gg in range(4):
        tiles += [(gg >> 1, gg & 1, T) for T in un_tiles[gg]]
    slots = int(sum(t[2] for t in tiles))

    # per-segment slot bases (order: P0, P1, [D0, D1,] U00, U01, U10, U11)
    seg_slot_base = []
    off = 0
    for gd in (0, 1):
        seg_slot_base.append(off)
        off += 2 * sum(pair_tiles[gd])
    d_slot_base = []
    for gs in range(len(pairs2_gs)):
        d_slot_base.append(off)
        off += 2 * sum(pair2_tiles[gs])
    for gg in range(4):
        seg_slot_base.append(off)
        off += sum(un_tiles[gg])

    wb = np.stack([np.full(P, w), np.full(P, b)], axis=1).astype(np.float32)
    in_maps = []
    s2e_list = []
    big = PAIR_T // 2
    for c in range(N_CORES):
        qs = np.zeros(slots, np.int16)
        qd = np.zeros(slots, np.int16)
        s2e = np.full(slots, -1, np.int64)
        for gd in (0, 1):
            g0, g1 = pairs_gd[gd]
            # contiguous q-chunk per core: near-sequential HBM reads for
            # this core's pair descriptors (lists are q-sorted)
            if CHUNK_DEAL:
                p0 = np.array_split(g0, N_CORES)[c]
                p1 = np.array_split(g1, N_CORES)[c]
            else:
                p0, p1 = g0[c::N_CORES], g1[c::N_CORES]
            base = seg_slot_base[gd]
            n = len(p0)
            j = np.arange(n)
            tile_of_j = j // big
            jt = j - tile_of_j * big
            # tile slot bases within the segment (tiles sized per plan)
            tsizes = np.asarray(pair_tiles[gd])
            tbase = np.concatenate([[0], np.cumsum(2 * tsizes)[:-1]])
            pt = jt % 128
            bt = 2 * (jt // 128)
            l0 = tbase[tile_of_j] + bt * 128 + pt
            l1 = l0 + 128
            # pair idx stream: position k within tile holds pair k's q
            ppos = tbase[tile_of_j] + jt
            qs[base + ppos] = (s[p0] >> 1).astype(np.int16)
            qd[base + l0] = (d[p0] >> 1).astype(np.int16)
            qd[base + l1] = (d[p1] >> 1).astype(np.int16)
            s2e[base + l0] = p0
            s2e[base + l1] = p1
        for gs in range(len(pairs2_gs)):
            g0, g1 = pairs2_gs[gs]
            if CHUNK_DEAL:
                p0 = np.array_split(g0, N_CORES)[c]
                p1 = np.array_split(g1, N_CORES)[c]
            else:
                p0, p1 = g0[c::N_CORES], g1[c::N_CORES]
            base = d_slot_base[gs]
            n = len(p0)
            j = np.arange(n)
            tile_of_j = j // big
            jt = j - tile_of_j * big
            tsizes = np.asarray(pair2_tiles[gs])
            tbase = np.concatenate([[0], np.cumsum(2 * tsizes)[:-1]])
            pt = jt % 128
            bt = 2 * (jt // 128)
            l0 = tbase[tile_of_j] + bt * 128 + pt
            l1 = l0 + 128
            ppos = tbase[tile_of_j] + jt
            qd[base + ppos] = (d[p0] >> 1).astype(np.int16)
            qs[base + l0] = (s[p0] >> 1).astype(np.int16)
            qs[base + l1] = (s[p1] >> 1).astype(np.int16)
            s2e[base + l0] = p0
            s2e[base + l1] = p1
        for gs, gd, eg in un_gg:
            e = np.array_split(eg, N_CORES)[c]
            base = seg_slot_base[2 + gs * 2 + gd]
            n = len(e)
            qs[base:base + n] = (s[e] >> 1).astype(np.int16)
            qd[base:base + n] = (d[e] >> 1).astype(np.int16)
            s2e[base:base + n] = e
        in_maps.append({
            "Z": Z,
            "isrc": np.ascontiguousarray(qs.reshape(-1, 16).T),
            "idst": np.ascontiguousarray(qd.reshape(-1, 16).T),
            "wb": wb,
        })
        s2e_list.append(s2e)
    return in_maps, s2e_list, tiles, slots, E, n_nodes


def _postprocess(results, s2e_list, E):
    Y = np.empty(E, np.float32)
    for c in range(N_CORES):
        yslots = results[c]["y"].T.ravel()
        s2e = s2e_list[c]
        valid = s2e >= 0
        Y[s2e[valid]] = yslots[valid]
    return Y[:, None]


def kernel(Z, edge_index, w, b, _trace=False, _trace_kwargs=None):
    in_maps, s2e_list, tiles, slots, E, n_nodes = _prepare(Z, edge_index, w, b)
    nc = bacc.Bacc("TRN2", num_swdge_queues=NUM_QUEUES,
                   dynamic_dma_scratch_size=DMA_SCRATCH)
    _build(nc, n_nodes, tiles, slots)
    _spread_gather_queues(nc)
    nc.finalize()
    run = run_bass_kernel_spmd(
        nc,
        in_maps,
        core_ids=list(range(N_CORES)),
        trace=_trace,
        **(_trace_kwargs or {}),
    )
    out = _postprocess(run.results, s2e_list, E)
    if _trace:
        return out, run
    return out
